# revision 13
# baseline (speedup 1.0000x reference)
"""Trainium2 Bass kernel for AnyGPT local-attention block (8 NeuronCores).

Sharding: (batch, seq-half) -> 8 shards of 1024 query tokens each; every core
gets a 256-token k/v halo (zero-padded at sequence start), so no collectives
are needed and the host gather is a pure concatenation.

Per-core pipeline (all matmuls in bf16, LayerNorm/softmax math in fp32):
  qT/kT = W^T-major projections ([H, tok] layout), v natural ([tok, H]) with a
  built-in ones column for softmax denominators; attention uses DIAGONAL
  128-query tiling: each 128-query tile attends exactly 3 aligned 128-key
  blocks (strict-lower triangle / full / upper-incl triangle in [key, query]
  layout), so only 384 scores per query are computed (vs 512 for 256-query
  tiling) and the middle block needs no mask. Units of 4 heads amortize
  per-instruction overhead: 12 score matmuls (parity-paired on PE row
  groups), one EXP (Scalar), one triangle-mask multiply (DVE), 12 ctx
  matmuls, two eviction casts (DVE + GpSimd) and one denominator DMA.
  Softmax is unnormalized (no max subtraction; scores are O(30)) with the
  denominator recovered from the ones row and divided into ctx via a rank-1
  selector-matmul broadcast before the output projection.
"""

import numpy as np
import ml_dtypes

import concourse.bass as bass
import concourse.mybir as mybir
import concourse.tile as tile
from concourse import bacc

F32 = mybir.dt.float32
BF16 = mybir.dt.bfloat16

B, S, H, NH, HD, WIN = 4, 2048, 1024, 16, 64, 256
P = 128
SQ = 1024          # queries per core
SE = SQ + WIN      # ext tokens (halo + queries)
KB = H // P        # 8 contraction blocks
NQT = SQ // P      # 8 query tiles of 128
LN_EPS = 1e-7
NCORES = 8

AF = mybir.ActivationFunctionType
ALU = mybir.AluOpType


def _bcast_ap(handle, n_part):
    """[D] DRAM vector -> [n_part, D] partition-broadcast AP (step 0)."""
    ap = handle[:]
    return bass.AP(tensor=ap.tensor, offset=ap.offset, ap=[[0, n_part]] + list(ap.ap))


def _dead_dim(ap, n, axis=1):
    """Insert a step-0 (broadcast) dim of size n at free-dim position axis."""
    dims = [list(d) for d in ap.ap]
    return bass.AP(tensor=ap.tensor, offset=ap.offset,
                   ap=dims[:axis] + [[0, n]] + dims[axis:])


def build_nc():
    nc = bacc.Bacc("TRN2", target_bir_lowering=False, debug=False)

    xq_h = nc.declare_dram_parameter("xq", [SQ, H], F32, isOutput=False)
    xT_h = nc.declare_dram_parameter("xT", [H, SE], BF16, isOutput=False)
    wqT_h = nc.declare_dram_parameter("wqT", [H, H], BF16, isOutput=False)
    wkT_h = nc.declare_dram_parameter("wkT", [H, H], BF16, isOutput=False)
    wvT_h = nc.declare_dram_parameter("wvT", [H, H], BF16, isOutput=False)
    woT_h = nc.declare_dram_parameter("woT", [H, H], BF16, isOutput=False)
    lnw_h = nc.declare_dram_parameter("lnw", [H], F32, isOutput=False)
    lnbbo_h = nc.declare_dram_parameter("lnbbo", [H], F32, isOutput=False)
    vones_h = nc.declare_dram_parameter("vones", [SE], BF16, isOutput=False)
    sel_h = nc.declare_dram_parameter("sel", [NH, KB, P], BF16, isOutput=False)
    mask2_h = nc.declare_dram_parameter("mask2", [P, 2, P], BF16, isOutput=False)
    out_h = nc.declare_dram_parameter("out", [SQ, H], F32, isOutput=True)

    with tile.TileContext(nc) as tc:
        _body(tc, nc, xq_h, xT_h, wqT_h, wkT_h, wvT_h, woT_h, lnw_h, lnbbo_h,
              vones_h, sel_h, mask2_h, out_h)
    nc.compile()
    return nc


def _body(tc, nc, xq_h, xT_h, wqT_h, wkT_h, wvT_h, woT_h, lnw_h, lnbbo_h,
          vones_h, sel_h, mask2_h, out_h):
    with (
        tc.tile_pool(name="const", bufs=1) as const,
        tc.tile_pool(name="big", bufs=1) as big,
        tc.tile_pool(name="wpool", bufs=20) as wpool,
        tc.tile_pool(name="work", bufs=3) as work,
        tc.tile_pool(name="lnpool", bufs=2) as lnpool,
    ):
        # residual parks in DRAM between LN (early) and the final add (late);
        # SBUF is too tight to hold 4 MB of fp32 for the whole kernel
        res_dram = nc.dram_tensor("res_dram", [SQ, H], F32)
        # ---- constants ----
        lnw_b = const.tile([P, H], F32)
        nc.sync.dma_start(lnw_b[:], _bcast_ap(lnw_h, P))
        lnbbo_b = const.tile([P, H], F32)
        nc.sync.dma_start(lnbbo_b[:], _bcast_ap(lnbbo_h, P))
        eps_t = const.tile([P, 1], F32)
        nc.vector.memset(eps_t[:], LN_EPS)
        sel_sb = const.tile([NH, KB, P], BF16)
        nc.sync.dma_start(sel_sb[:], sel_h[:][:, :, :])
        # triangle masks in [key, query] layout: slot 0 = strictly-lower
        # (key > query, for the jb=0 block), slot 1 = upper-incl (key <=
        # query, for the jb=2 block); the jb=1 block is fully in-band
        mask2_sb = const.tile([P, 2, P], BF16)
        nc.sync.dma_start(mask2_sb[:], mask2_h[:][:, :, :])

        # ---- x^T resident [128, kb, tok]; the first q-proj PSUM group only
        # needs wq cols 0:128 + xt cols WIN:WIN+512, so those DMAs go first
        # and the first matmul starts several us earlier ----
        xt_sb = big.tile([P, KB, SE], BF16, tag="xtr")
        wq_sl = [wpool.tile([P, H], BF16, tag="wslice", name=f"wq_{kb}")
                 for kb in range(KB)]
        for kb in range(KB):
            nc.sync.dma_start(wq_sl[kb][:, 0:P],
                              wqT_h[:][kb * P:(kb + 1) * P, 0:P])
            nc.sync.dma_start(xt_sb[:, kb, WIN:WIN + 512],
                              xT_h[:][kb * P:(kb + 1) * P, WIN:WIN + 512])
        for kb in range(KB):
            nc.sync.dma_start(wq_sl[kb][:, P:],
                              wqT_h[:][kb * P:(kb + 1) * P, P:])
        for kb in range(KB):
            nc.sync.dma_start(xt_sb[:, kb, :WIN],
                              xT_h[:][kb * P:(kb + 1) * P, :WIN])
            nc.sync.dma_start(xt_sb[:, kb, WIN + 512:],
                              xT_h[:][kb * P:(kb + 1) * P, WIN + 512:])

        qT_sb = big.tile([P, KB, SQ], BF16)    # q^T  [H, 1024]
        kT_sb = big.tile([P, KB, SE], BF16)    # k^T  [H, 1280]
        # v natural + a "ones" column that is 0.0 for zero-padded halo tokens,
        # so pad keys contribute exactly nothing to ctx or the denominators
        v_sb = big.tile([P, SE // P, NH, HD + 1], BF16)
        vo = vones_h[:]
        vo_pt = bass.AP(tensor=vo.tensor, offset=vo.offset,
                        ap=[[1, P], [P, SE // P]])
        for h in range(NH):
            nc.sync.dma_start(v_sb[:, :, h, HD], vo_pt)
        ct_sb = big.tile([P, KB, SQ], BF16)    # UNnormalized ctx^T [H, 1024]
        den_sb = big.tile([NH, SQ], F32)       # softmax denominators [head, i]
        recip_sb = big.tile([NH, SQ], BF16)    # 1/den, bulk-reciprocated

        with tc.tile_pool(name="ppsum", bufs=4, space="PSUM") as ppsum:
            # wk slices requested right behind wq/xt so the DMA queues have
            # them in flight well before the k-projection starts
            wk_sl = [wpool.tile([P, H], BF16, tag="wslice", name=f"wk_{kb}")
                     for kb in range(KB)]
            for kb in range(KB):
                nc.sync.dma_start(wk_sl[kb][:], wkT_h[:][kb * P:(kb + 1) * P, :])

            # ---- transposed projections: q^T, k^T ----
            for (w_h, dst, tok0, ntok, wsl) in ((wqT_h, qT_sb, WIN, SQ, wq_sl),
                                                (wkT_h, kT_sb, 0, SE, wk_sl)):
                chunks = [(i, min(512, ntok - i)) for i in range(0, ntok, 512)]
                for ob in range(KB):
                    for (i0, ilen) in chunks:
                        ps = ppsum.tile([P, 512], F32, tag="pj", name="ps_qk")
                        for kb in range(KB):
                            nc.tensor.matmul(
                                ps[:, :ilen],
                                wsl[kb][:, ob * P:(ob + 1) * P],
                                xt_sb[:, kb, tok0 + i0: tok0 + i0 + ilen],
                                start=(kb == 0), stop=(kb == KB - 1),
                            )
                        nc.scalar.copy(out=dst[:, ob, i0:i0 + ilen],
                                       in_=ps[:, :ilen])

            # ---- natural projection: v ----
            wsl = [wpool.tile([P, H], BF16, tag="wslice", name=f"wv_{kb}")
                   for kb in range(KB)]
            for kb in range(KB):
                nc.sync.dma_start(wsl[kb][:], wvT_h[:][kb * P:(kb + 1) * P, :])
            for tt in range(SE // P):
                for oh in range(2):
                    ps = ppsum.tile([P, 512], F32, tag="pj", name="ps_v")
                    for kb in range(KB):
                        nc.tensor.matmul(
                            ps[:],
                            xt_sb[:, kb, tt * P:(tt + 1) * P],
                            wsl[kb][:, oh * 512:(oh + 1) * 512],
                            start=(kb == 0), stop=(kb == KB - 1),
                        )
                    nc.scalar.copy(
                        out=v_sb[:, tt, oh * 8:(oh + 1) * 8, 0:HD],
                        in_=ps[:].rearrange("p (h d) -> p h d", d=HD),
                    )

            # ---- wo slices (prefetch; consumed at the end) ----
            wosl = [wpool.tile([P, H], BF16, tag="wslice", name=f"wo_{kb}")
                    for kb in range(KB)]
            for kb in range(KB):
                nc.sync.dma_start(wosl[kb][:], woT_h[:][kb * P:(kb + 1) * P, :])

            # ---- LayerNorm residual (DVE+GpSimd are idle during
            # projections; the affine tail is split across both) ----
            for it in range(KB):
                x_t = lnpool.tile([P, H], F32, tag="x_t", name="x_t")
                nc.sync.dma_start(x_t[:], xq_h[:][it * P:(it + 1) * P, :])
                stats = lnpool.tile([P, 2, 6], F32, tag="stats", name="stats")
                for g in range(2):
                    nc.vector.bn_stats(out=stats[:, g, :],
                                       in_=x_t[:, g * 512:(g + 1) * 512])
                mv = lnpool.tile([P, 2], F32, tag="mv", name="mv")
                nc.vector.bn_aggr(out=mv[:], in_=stats[:])
                std = lnpool.tile([P, 1], F32, tag="std", name="std")
                nc.scalar.activation(out=std[:], in_=mv[:, 1:2], func=AF.Sqrt,
                                     bias=eps_t[:])
                rstd = lnpool.tile([P, 1], F32, tag="rstd", name="rstd")
                nc.vector.reciprocal(out=rstd[:], in_=std[:])
                res_t = lnpool.tile([P, H], F32, tag="res_t", name="res_t")
                nc.vector.tensor_scalar(out=res_t[:], in0=x_t[:],
                                        scalar1=mv[:, 0:1], scalar2=rstd[:],
                                        op0=ALU.subtract, op1=ALU.mult)
                # affine tail on GpSimd (plain tensor_tensor only — the ISA
                # rejects scalar_tensor_tensor on Pool), freeing DVE
                nc.gpsimd.tensor_mul(out=res_t[:], in0=res_t[:], in1=lnw_b[:])
                nc.gpsimd.tensor_add(out=res_t[:], in0=res_t[:], in1=lnbbo_b[:])
                nc.sync.dma_start(res_dram[it * P:(it + 1) * P, :], res_t[:])

        # ---- attention: diagonal 128-query tiling, 4 heads per unit ----
        # Query tile t attends exactly key blocks t, t+1, t+2 of the ext
        # sequence (jb=0 strict-lower triangle, jb=1 full, jb=2 upper-incl
        # triangle in [key, query] layout). Score matmuls parity-pair on PE
        # row groups (even head rows 0-63, odd rows 64-127). Software
        # pipeline: scores lookahead 2 units, exp lookahead 1.
        with (
            tc.tile_pool(name="spsum", bufs=2, space="PSUM") as spsum,
        ):
            units = [(t, hq) for t in range(NQT) for hq in range(4)]
            sp_of, probs_of = {}, {}

            def emit_scores(i):
                t, hq = units[i]
                # one 4-bank PSUM tile per unit: parity stride = one full
                # bank (tile-position-paired matmuls drain concurrently and
                # concurrent drains into one bank are fatal). The jb
                # padding slot 3 doubles as the ctx-matmul output region,
                # so no separate ctx PSUM pool is needed.
                sp = spsum.tile([P, 4, 4, P], F32, tag="sc", name="sp")
                for jb in range(3):
                    ks = slice((t + jb) * P, (t + jb + 1) * P)
                    for p in range(4):
                        hb = 2 * hq + p // 2
                        ho = (p % 2) * HD
                        nc.tensor.matmul(
                            sp[:, p, jb, :],
                            kT_sb[ho:ho + HD, hb, ks],
                            qT_sb[ho:ho + HD, hb, t * P:(t + 1) * P],
                            start=True, stop=True,
                        )
                sp_of[i] = sp

            def emit_probs(i):
                sp = sp_of[i]
                probs = work.tile([P, 4, 3, P], BF16, tag="probs",
                                  name="probs", bufs=3)
                nc.scalar.activation(out=probs[:], in_=sp[:, :, 0:3, :],
                                     func=AF.Exp)
                # triangle masks on the jb=0 / jb=2 blocks in one multiply;
                # GpSimd is idle and this is SBUF-to-SBUF (legal there)
                nc.vector.tensor_mul(
                    out=probs[:, :, 0:3:2, :], in0=probs[:, :, 0:3:2, :],
                    in1=_dead_dim(mask2_sb[:], 4))
                probs_of[i] = probs

            def emit_ctx(i):
                t, hq = units[i]
                probs = probs_of.pop(i)
                sp = sp_of.pop(i)
                # ctx accumulates into the pad slot (jb=3) of this unit's
                # own scores tile: safely ordered after the exp read via
                # the probs dependency
                for p in range(4):
                    for jb in range(3):
                        nc.tensor.matmul(
                            sp[0:HD + 1, p, 3, :],
                            v_sb[:, t + jb, 4 * hq + p, :],
                            probs[:, p, jb, :],
                            start=(jb == 0), stop=(jb == 2),
                        )
                tq = slice(t * P, (t + 1) * P)
                # evictions on DVE (GpSimd cannot read PSUM)
                nc.vector.tensor_copy(out=ct_sb[0:HD, 2 * hq:2 * hq + 2, tq],
                                      in_=sp[0:HD, 0:4:2, 3, :])
                nc.vector.tensor_copy(out=ct_sb[HD:P, 2 * hq:2 * hq + 2, tq],
                                      in_=sp[0:HD, 1:4:2, 3, :])
                dstage = work.tile([1, 4, P], F32, tag="dstage",
                                   name="dstage", bufs=3)
                # denominator staging alternates DVE/Scalar to balance load
                eng = nc.vector if i % 2 == 0 else nc.scalar
                if eng is nc.vector:
                    nc.vector.tensor_copy(out=dstage[:],
                                          in_=sp[HD:HD + 1, :, 3, :])
                else:
                    nc.scalar.copy(out=dstage[:], in_=sp[HD:HD + 1, :, 3, :])
                nc.sync.dma_start(out=den_sb[4 * hq:4 * hq + 4, tq],
                                  in_=dstage[:])
                if hq == 3:
                    # all heads of this query tile done: reciprocate its
                    # denominator slice now so out-proj never waits on it
                    with nc.allow_low_precision(
                            reason="softmax denom recip in bf16: 0.4% rel "
                                   "on a 2e-2 budget"):
                        nc.vector.reciprocal(out=recip_sb[:, tq],
                                             in_=den_sb[:, tq])

            emit_scores(0)
            emit_scores(1)
            emit_probs(0)
            for i in range(len(units)):
                if i + 2 < len(units):
                    emit_scores(i + 2)
                if i + 1 < len(units):
                    emit_probs(i + 1)
                emit_ctx(i)

        # ---- normalize ctx^T, then output projection + residual ----
        # R = selector-matmul broadcast of the per-head reciprocals into the
        # [128, 128] block layout of ct_sb (rows 0-63 <- even head, 64-127 <-
        # odd head), then ct_sb *= R in place.
        with (
            tc.tile_pool(name="opsum", bufs=4, space="PSUM") as opsum,
            tc.tile_pool(name="rpsum", bufs=4, space="PSUM") as rpsum,
        ):
            # R broadcast in bulk: 2 big selector-matmuls per h-block, evicted
            # to SBUF by the otherwise-idle scalar engine, then wide bf16 DVE
            # multiplies normalize ct in place (no per-tile PSUM chain).
            r_sb = big.tile([P, KB, SQ], BF16, tag="xtr")
            resld = {}
            for it in range(2):
                for oh in range(2):
                    t = work.tile([P, 512], F32, tag="resld", name="resld",
                                  bufs=4)
                    nc.sync.dma_start(t[:], res_dram[it * P:(it + 1) * P,
                                                     oh * 512:(oh + 1) * 512])
                    resld[(it, oh)] = t
            for ih in range(2):
                # normalize ct for this i-half first (R broadcast + wide DVE
                # muls), then immediately run its 4 out-proj row-tiles
                hsl = slice(ih * 512, (ih + 1) * 512)
                for hb in range(KB):
                    ps_rb = rpsum.tile([P, 512], F32, tag="rb", name="ps_rb")
                    nc.tensor.matmul(ps_rb[:], sel_sb[:, hb, :],
                                     recip_sb[:, hsl], start=True, stop=True)
                    nc.scalar.copy(out=r_sb[:, hb, hsl], in_=ps_rb[:])
                    nc.vector.tensor_mul(out=ct_sb[:, hb, hsl],
                                         in0=ct_sb[:, hb, hsl],
                                         in1=r_sb[:, hb, hsl])
                for it in range(4 * ih, 4 * ih + 4):
                    for oh in range(2):
                        if it + 2 < KB:  # prefetch 2 row-tiles ahead
                            t = work.tile([P, 512], F32, tag="resld",
                                          name="resld", bufs=4)
                            nc.sync.dma_start(
                                t[:], res_dram[(it + 2) * P:(it + 3) * P,
                                               oh * 512:(oh + 1) * 512])
                            resld[(it + 2, oh)] = t
                        ps_o = opsum.tile([P, 512], F32, tag="po", name="ps_o")
                        for hb in range(KB):
                            nc.tensor.matmul(
                                ps_o[:],
                                ct_sb[:, hb, it * P:(it + 1) * P],
                                wosl[hb][:, oh * 512:(oh + 1) * 512],
                                start=(hb == 0), stop=(hb == KB - 1),
                            )
                        o_t = work.tile([P, 512], F32, tag="o_t", name="o_t",
                                        bufs=2)
                        nc.vector.tensor_add(out=o_t[:], in0=ps_o[:],
                                             in1=resld.pop((it, oh))[:])
                        nc.sync.dma_start(
                            out_h[:][it * P:(it + 1) * P,
                                     oh * 512:(oh + 1) * 512],
                            o_t[:])


_CACHE = {}


def get_nc():
    if "nc" not in _CACHE:
        _CACHE["nc"] = build_nc()
    return _CACHE["nc"]


def make_in_maps(inputs):
    x = np.asarray(inputs["hidden_states"], dtype=np.float32)
    wq = np.asarray(inputs["wq"], dtype=np.float32)
    wk = np.asarray(inputs["wk"], dtype=np.float32)
    wv = np.asarray(inputs["wv"], dtype=np.float32)
    wo = np.asarray(inputs["wo"], dtype=np.float32)
    bo = np.asarray(inputs["bo"], dtype=np.float32)
    ln_w = np.asarray(inputs["ln_w"], dtype=np.float32)
    ln_b = np.asarray(inputs["ln_b"], dtype=np.float32)

    bf = ml_dtypes.bfloat16
    wqT = np.ascontiguousarray(wq.T).astype(bf)
    wkT = np.ascontiguousarray(wk.T).astype(bf)
    wvT = np.ascontiguousarray(wv.T).astype(bf)
    woT = np.ascontiguousarray(wo.T).astype(bf)
    lnbbo = (ln_b + bo).astype(np.float32)

    # selector for the reciprocal broadcast: sel[p, hb, m] = 1 iff head p owns
    # row m of h-block hb in the ct layout (even head -> rows 0-63, odd -> 64+)
    sel = np.zeros((NH, KB, P), dtype=np.float32)
    for hb in range(KB):
        sel[2 * hb, hb, :HD] = 1.0
        sel[2 * hb + 1, hb, HD:] = 1.0
    sel = sel.astype(bf)

    # triangle masks in [key, query] layout (see _body)
    r = np.arange(P)[:, None]
    c = np.arange(P)[None, :]
    mask2 = np.stack([(r > c), (r <= c)], axis=1).astype(bf)  # [P, 2, P]

    in_maps = []
    for core in range(NCORES):
        b, hh = divmod(core, 2)
        start = hh * SQ
        xkv = np.zeros((SE, H), dtype=np.float32)
        xkv[WIN:] = x[b, start:start + SQ]
        vones = np.ones(SE, dtype=np.float32)
        if start > 0:
            xkv[:WIN] = x[b, start - WIN:start]
        else:
            vones[:WIN] = 0.0
        in_maps.append({
            "xq": np.ascontiguousarray(x[b, start:start + SQ]),
            "xT": np.ascontiguousarray(xkv.T).astype(bf),
            "wqT": wqT, "wkT": wkT, "wvT": wvT, "woT": woT,
            "lnw": ln_w, "lnbbo": lnbbo,
            "vones": vones.astype(bf),
            "sel": sel,
            "mask2": mask2,
        })
    return in_maps


def kernel(**inputs):
    from concourse.bass_utils import run_bass_kernel_spmd
    nc = get_nc()
    in_maps = make_in_maps(inputs)
    res = run_bass_kernel_spmd(nc, in_maps, core_ids=list(range(NCORES)))
    out = np.empty((B, S, H), dtype=np.float32)
    for core in range(NCORES):
        b, hh = divmod(core, 2)
        out[b, hh * SQ:(hh + 1) * SQ, :] = res.results[core]["out"]
    return out


# revision 14
# speedup vs baseline: 1.2295x; 1.2295x over previous
"""Trainium2 Bass kernel for AnyGPT local-attention block (8 NeuronCores).

Sharding: (batch, seq-half) -> 8 shards of 1024 query tokens each; every core
gets a 256-token k/v halo (zero-padded at sequence start), so no collectives
are needed and the host gather is a pure concatenation.

Per-core pipeline (all matmuls in bf16, LayerNorm/softmax math in fp32):
  qT/kT = W^T-major projections ([H, tok] layout), v natural ([tok, H]) with a
  built-in ones column for softmax denominators; attention uses DIAGONAL
  128-query tiling: each 128-query tile attends exactly 3 aligned 128-key
  blocks (strict-lower triangle / full / upper-incl triangle in [key, query]
  layout), so only 384 scores per query are computed (vs 512 for 256-query
  tiling) and the middle block needs no mask. Units of 4 heads amortize
  per-instruction overhead: 12 score matmuls (parity-paired on PE row
  groups), one EXP (Scalar), one triangle-mask multiply (DVE), 12 ctx
  matmuls, two eviction casts (DVE + GpSimd) and one denominator DMA.
  Softmax is unnormalized (no max subtraction; scores are O(30)) with the
  denominator recovered from the ones row and divided into ctx via a rank-1
  selector-matmul broadcast before the output projection.
"""

import numpy as np
import ml_dtypes

import concourse.bass as bass
import concourse.mybir as mybir
import concourse.tile as tile
from concourse import bacc

F32 = mybir.dt.float32
BF16 = mybir.dt.bfloat16

B, S, H, NH, HD, WIN = 4, 2048, 1024, 16, 64, 256
P = 128
SQ = 1024          # queries per core
SE = SQ + WIN      # ext tokens (halo + queries)
KB = H // P        # 8 contraction blocks
NQT = SQ // P      # 8 query tiles of 128
LN_EPS = 1e-7
NCORES = 8

AF = mybir.ActivationFunctionType
ALU = mybir.AluOpType


def _bcast_ap(handle, n_part):
    """[D] DRAM vector -> [n_part, D] partition-broadcast AP (step 0)."""
    ap = handle[:]
    return bass.AP(tensor=ap.tensor, offset=ap.offset, ap=[[0, n_part]] + list(ap.ap))


def _dead_dim(ap, n, axis=1):
    """Insert a step-0 (broadcast) dim of size n at free-dim position axis."""
    dims = [list(d) for d in ap.ap]
    return bass.AP(tensor=ap.tensor, offset=ap.offset,
                   ap=dims[:axis] + [[0, n]] + dims[axis:])


def build_nc():
    nc = bacc.Bacc("TRN2", target_bir_lowering=False, debug=False)

    xq_h = nc.declare_dram_parameter("xq", [SQ, H], F32, isOutput=False)
    xT_h = nc.declare_dram_parameter("xT", [H, SE], BF16, isOutput=False)
    wqT_h = nc.declare_dram_parameter("wqT", [H, H], BF16, isOutput=False)
    wkT_h = nc.declare_dram_parameter("wkT", [H, H], BF16, isOutput=False)
    wvT_h = nc.declare_dram_parameter("wvT", [H, H], BF16, isOutput=False)
    woT_h = nc.declare_dram_parameter("woT", [H, H], BF16, isOutput=False)
    lnw_h = nc.declare_dram_parameter("lnw", [H], F32, isOutput=False)
    lnbbo_h = nc.declare_dram_parameter("lnbbo", [H], F32, isOutput=False)
    vones_h = nc.declare_dram_parameter("vones", [SE], BF16, isOutput=False)
    sel_h = nc.declare_dram_parameter("sel", [NH, KB, P], BF16, isOutput=False)
    mask2_h = nc.declare_dram_parameter("mask2", [P, 2, P], BF16, isOutput=False)
    out_h = nc.declare_dram_parameter("out", [SQ, H], F32, isOutput=True)

    with tile.TileContext(nc) as tc:
        _body(tc, nc, xq_h, xT_h, wqT_h, wkT_h, wvT_h, woT_h, lnw_h, lnbbo_h,
              vones_h, sel_h, mask2_h, out_h)
    nc.compile()
    return nc


def _body(tc, nc, xq_h, xT_h, wqT_h, wkT_h, wvT_h, woT_h, lnw_h, lnbbo_h,
          vones_h, sel_h, mask2_h, out_h):
    with (
        tc.tile_pool(name="const", bufs=1) as const,
        tc.tile_pool(name="big", bufs=1) as big,
        tc.tile_pool(name="wpool", bufs=20) as wpool,
        tc.tile_pool(name="work", bufs=3) as work,
        tc.tile_pool(name="lnpool", bufs=2) as lnpool,
    ):
        # residual parks in DRAM between LN (early) and the final add (late);
        # SBUF is too tight to hold 4 MB of fp32 for the whole kernel
        res_dram = nc.dram_tensor("res_dram", [SQ, H], F32)
        # ---- constants ----
        lnw_b = const.tile([P, H], F32)
        nc.sync.dma_start(lnw_b[:], _bcast_ap(lnw_h, P))
        lnbbo_b = const.tile([P, H], F32)
        nc.sync.dma_start(lnbbo_b[:], _bcast_ap(lnbbo_h, P))
        eps_t = const.tile([P, 1], F32)
        nc.vector.memset(eps_t[:], LN_EPS)
        sel_sb = const.tile([NH, KB, P], BF16)
        nc.sync.dma_start(sel_sb[:], sel_h[:][:, :, :])
        # triangle masks in [key, query] layout: slot 0 = strictly-lower
        # (key > query, for the jb=0 block), slot 1 = upper-incl (key <=
        # query, for the jb=2 block); the jb=1 block is fully in-band
        mask2_sb = const.tile([P, 2, P], BF16)
        nc.sync.dma_start(mask2_sb[:], mask2_h[:][:, :, :])

        # ---- x^T resident [128, kb, tok]; the first q-proj PSUM group only
        # needs wq cols 0:128 + xt cols WIN:WIN+512, so those DMAs go first
        # and the first matmul starts several us earlier ----
        xt_sb = big.tile([P, KB, SE], BF16, tag="xtr")
        wq_sl = [wpool.tile([P, H], BF16, tag="wslice", name=f"wq_{kb}")
                 for kb in range(KB)]
        for kb in range(KB):
            nc.sync.dma_start(wq_sl[kb][:, 0:P],
                              wqT_h[:][kb * P:(kb + 1) * P, 0:P])
            nc.sync.dma_start(xt_sb[:, kb, WIN:WIN + 512],
                              xT_h[:][kb * P:(kb + 1) * P, WIN:WIN + 512])
        for kb in range(KB):
            nc.sync.dma_start(wq_sl[kb][:, P:],
                              wqT_h[:][kb * P:(kb + 1) * P, P:])
        for kb in range(KB):
            nc.sync.dma_start(xt_sb[:, kb, :WIN],
                              xT_h[:][kb * P:(kb + 1) * P, :WIN])
            nc.sync.dma_start(xt_sb[:, kb, WIN + 512:],
                              xT_h[:][kb * P:(kb + 1) * P, WIN + 512:])

        qT_sb = big.tile([P, KB, SQ], BF16)    # q^T  [H, 1024]
        kT_sb = big.tile([P, KB, SE], BF16)    # k^T  [H, 1280]
        # v natural + a "ones" column that is 0.0 for zero-padded halo tokens,
        # so pad keys contribute exactly nothing to ctx or the denominators
        v_sb = big.tile([P, SE // P, NH, HD + 1], BF16)
        vo = vones_h[:]
        vo_pt = bass.AP(tensor=vo.tensor, offset=vo.offset,
                        ap=[[1, P], [P, SE // P]])
        for h in range(NH):
            nc.sync.dma_start(v_sb[:, :, h, HD], vo_pt)
        ct_sb = big.tile([P, KB, SQ], BF16)    # UNnormalized ctx^T [H, 1024]
        den_sb = big.tile([NH, SQ], F32)       # softmax denominators [head, i]
        recip_sb = big.tile([NH, SQ], BF16)    # 1/den, bulk-reciprocated

        with tc.tile_pool(name="ppsum", bufs=4, space="PSUM") as ppsum:
            # wk slices requested right behind wq/xt so the DMA queues have
            # them in flight well before the k-projection starts
            wk_sl = [wpool.tile([P, H], BF16, tag="wslice", name=f"wk_{kb}")
                     for kb in range(KB)]
            for kb in range(KB):
                nc.sync.dma_start(wk_sl[kb][:], wkT_h[:][kb * P:(kb + 1) * P, :])

            # ---- transposed projections: q^T, k^T ----
            for (w_h, dst, tok0, ntok, wsl) in ((wqT_h, qT_sb, WIN, SQ, wq_sl),
                                                (wkT_h, kT_sb, 0, SE, wk_sl)):
                chunks = [(i, min(512, ntok - i)) for i in range(0, ntok, 512)]
                for ob in range(KB):
                    for (i0, ilen) in chunks:
                        ps = ppsum.tile([P, 512], F32, tag="pj", name="ps_qk")
                        for kb in range(KB):
                            nc.tensor.matmul(
                                ps[:, :ilen],
                                wsl[kb][:, ob * P:(ob + 1) * P],
                                xt_sb[:, kb, tok0 + i0: tok0 + i0 + ilen],
                                start=(kb == 0), stop=(kb == KB - 1),
                            )
                        nc.scalar.copy(out=dst[:, ob, i0:i0 + ilen],
                                       in_=ps[:, :ilen])

            # ---- natural projection: v ----
            wsl = [wpool.tile([P, H], BF16, tag="wslice", name=f"wv_{kb}")
                   for kb in range(KB)]
            for kb in range(KB):
                nc.sync.dma_start(wsl[kb][:], wvT_h[:][kb * P:(kb + 1) * P, :])
            for tt in range(SE // P):
                for oh in range(2):
                    ps = ppsum.tile([P, 512], F32, tag="pj", name="ps_v")
                    for kb in range(KB):
                        nc.tensor.matmul(
                            ps[:],
                            xt_sb[:, kb, tt * P:(tt + 1) * P],
                            wsl[kb][:, oh * 512:(oh + 1) * 512],
                            start=(kb == 0), stop=(kb == KB - 1),
                        )
                    nc.scalar.copy(
                        out=v_sb[:, tt, oh * 8:(oh + 1) * 8, 0:HD],
                        in_=ps[:].rearrange("p (h d) -> p h d", d=HD),
                    )

            # ---- wo slices (prefetch; consumed at the end) ----
            wosl = [wpool.tile([P, H], BF16, tag="wslice", name=f"wo_{kb}")
                    for kb in range(KB)]
            for kb in range(KB):
                nc.sync.dma_start(wosl[kb][:], woT_h[:][kb * P:(kb + 1) * P, :])

            # ---- LayerNorm residual (DVE+GpSimd are idle during
            # projections; the affine tail is split across both) ----
            for it in range(KB):
                x_t = lnpool.tile([P, H], F32, tag="x_t", name="x_t")
                nc.sync.dma_start(x_t[:], xq_h[:][it * P:(it + 1) * P, :])
                stats = lnpool.tile([P, 2, 6], F32, tag="stats", name="stats")
                for g in range(2):
                    nc.vector.bn_stats(out=stats[:, g, :],
                                       in_=x_t[:, g * 512:(g + 1) * 512])
                mv = lnpool.tile([P, 2], F32, tag="mv", name="mv")
                nc.vector.bn_aggr(out=mv[:], in_=stats[:])
                std = lnpool.tile([P, 1], F32, tag="std", name="std")
                nc.scalar.activation(out=std[:], in_=mv[:, 1:2], func=AF.Sqrt,
                                     bias=eps_t[:])
                rstd = lnpool.tile([P, 1], F32, tag="rstd", name="rstd")
                nc.vector.reciprocal(out=rstd[:], in_=std[:])
                res_t = lnpool.tile([P, H], F32, tag="res_t", name="res_t")
                nc.vector.tensor_scalar(out=res_t[:], in0=x_t[:],
                                        scalar1=mv[:, 0:1], scalar2=rstd[:],
                                        op0=ALU.subtract, op1=ALU.mult)
                # affine tail on GpSimd (plain tensor_tensor only — the ISA
                # rejects scalar_tensor_tensor on Pool), freeing DVE
                nc.gpsimd.tensor_mul(out=res_t[:], in0=res_t[:], in1=lnw_b[:])
                nc.gpsimd.tensor_add(out=res_t[:], in0=res_t[:], in1=lnbbo_b[:])
                nc.sync.dma_start(res_dram[it * P:(it + 1) * P, :], res_t[:])

        # ---- attention: diagonal 128-query tiling, 4 heads per unit ----
        # Query tile t attends exactly key blocks t, t+1, t+2 of the ext
        # sequence (jb=0 strict-lower triangle, jb=1 full, jb=2 upper-incl
        # triangle in [key, query] layout). Score matmuls parity-pair on PE
        # row groups (even head rows 0-63, odd rows 64-127). Software
        # pipeline: scores lookahead 2 units, exp lookahead 1.
        with (
            tc.tile_pool(name="spsum", bufs=2, space="PSUM") as spsum,
            tc.tile_pool(name="cpsum", bufs=2, space="PSUM") as cpsum,
        ):
            units = [(t, hq) for t in range(NQT) for hq in range(4)]
            sp_of, probs_of = {}, {}

            def emit_scores(i):
                t, hq = units[i]
                # one 2-bank PSUM tile per head-PAIR, parity stride = one
                # full bank: tile-position-paired matmuls drain concurrently
                # and concurrent drains into one bank are fatal. jb dim is
                # padded 3->4 so each parity owns exactly one bank.
                spA = spsum.tile([P, 2, 4, P], F32, tag="scA", name="spA")
                spB = spsum.tile([P, 2, 4, P], F32, tag="scB", name="spB",
                                 bufs=1)
                for jb in range(3):
                    ks = slice((t + jb) * P, (t + jb + 1) * P)
                    for p in range(4):
                        hb = 2 * hq + p // 2
                        ho = (p % 2) * HD
                        sp = spA if p < 2 else spB
                        nc.tensor.matmul(
                            sp[:, p % 2, jb, :],
                            kT_sb[ho:ho + HD, hb, ks],
                            qT_sb[ho:ho + HD, hb, t * P:(t + 1) * P],
                            start=True, stop=True,
                        )
                sp_of[i] = (spA, spB)

            def emit_probs(i):
                spA, spB = sp_of.pop(i)
                probs = work.tile([P, 4, 3, P], BF16, tag="probs",
                                  name="probs", bufs=3)
                nc.scalar.activation(out=probs[:, 2:4, :, :],
                                     in_=spB[:, :, 0:3, :], func=AF.Exp)
                nc.scalar.activation(out=probs[:, 0:2, :, :],
                                     in_=spA[:, :, 0:3, :], func=AF.Exp)
                # triangle masks on the jb=0 / jb=2 blocks in one multiply;
                # GpSimd (SBUF-to-SBUF, so legal there) — DVE is saturated
                # with PSUM evictions and GpSimd cannot read PSUM
                nc.vector.tensor_mul(
                    out=probs[:, :, 0:3:2, :], in0=probs[:, :, 0:3:2, :],
                    in1=_dead_dim(mask2_sb[:], 4))
                probs_of[i] = probs

            def emit_ctx(i):
                t, hq = units[i]
                probs = probs_of.pop(i)
                pc = cpsum.tile([HD + 1, 4, P], F32, tag="cx", name="pc")
                for p in range(4):
                    for jb in range(3):
                        nc.tensor.matmul(
                            pc[:, p, :],
                            v_sb[:, t + jb, 4 * hq + p, :],
                            probs[:, p, jb, :],
                            start=(jb == 0), stop=(jb == 2),
                        )
                tq = slice(t * P, (t + 1) * P)
                # evictions all on DVE (GpSimd cannot read PSUM)
                nc.vector.tensor_copy(out=ct_sb[0:HD, 2 * hq:2 * hq + 2, tq],
                                      in_=pc[0:HD, 0:4:2, :])
                nc.vector.tensor_copy(out=ct_sb[HD:P, 2 * hq:2 * hq + 2, tq],
                                      in_=pc[0:HD, 1:4:2, :])
                dstage = work.tile([1, 4, P], F32, tag="dstage",
                                   name="dstage", bufs=3)
                # denominator staging alternates DVE/Scalar to balance load
                if i % 2 == 0:
                    nc.vector.tensor_copy(out=dstage[:],
                                          in_=pc[HD:HD + 1, :, :])
                else:
                    nc.scalar.copy(out=dstage[:], in_=pc[HD:HD + 1, :, :])
                nc.sync.dma_start(out=den_sb[4 * hq:4 * hq + 4, tq],
                                  in_=dstage[:])
                if hq == 3:
                    # all heads of this query tile done: reciprocate its
                    # denominator slice now so out-proj never waits on it
                    with nc.allow_low_precision(
                            reason="softmax denom recip in bf16: 0.4% rel "
                                   "on a 2e-2 budget"):
                        nc.vector.reciprocal(out=recip_sb[:, tq],
                                             in_=den_sb[:, tq])

            emit_scores(0)
            emit_scores(1)
            emit_probs(0)
            for i in range(len(units)):
                if i + 2 < len(units):
                    emit_scores(i + 2)
                if i + 1 < len(units):
                    emit_probs(i + 1)
                emit_ctx(i)

        # ---- normalize ctx^T, then output projection + residual ----
        # R = selector-matmul broadcast of the per-head reciprocals into the
        # [128, 128] block layout of ct_sb (rows 0-63 <- even head, 64-127 <-
        # odd head), then ct_sb *= R in place.
        with (
            tc.tile_pool(name="opsum", bufs=4, space="PSUM") as opsum,
            tc.tile_pool(name="rpsum", bufs=4, space="PSUM") as rpsum,
        ):
            # R broadcast in bulk: 2 big selector-matmuls per h-block, evicted
            # to SBUF by the otherwise-idle scalar engine, then wide bf16 DVE
            # multiplies normalize ct in place (no per-tile PSUM chain).
            r_sb = big.tile([P, KB, SQ], BF16, tag="xtr")
            resld = {}
            for it in range(2):
                for oh in range(2):
                    t = work.tile([P, 512], F32, tag="resld", name="resld",
                                  bufs=4)
                    nc.sync.dma_start(t[:], res_dram[it * P:(it + 1) * P,
                                                     oh * 512:(oh + 1) * 512])
                    resld[(it, oh)] = t
            for ih in range(2):
                # normalize ct for this i-half first (R broadcast + wide DVE
                # muls), then immediately run its 4 out-proj row-tiles
                hsl = slice(ih * 512, (ih + 1) * 512)
                for hb in range(KB):
                    ps_rb = rpsum.tile([P, 512], F32, tag="rb", name="ps_rb")
                    nc.tensor.matmul(ps_rb[:], sel_sb[:, hb, :],
                                     recip_sb[:, hsl], start=True, stop=True)
                    nc.scalar.copy(out=r_sb[:, hb, hsl], in_=ps_rb[:])
                    nc.vector.tensor_mul(out=ct_sb[:, hb, hsl],
                                         in0=ct_sb[:, hb, hsl],
                                         in1=r_sb[:, hb, hsl])
                for it in range(4 * ih, 4 * ih + 4):
                    for oh in range(2):
                        if it + 2 < KB:  # prefetch 2 row-tiles ahead
                            t = work.tile([P, 512], F32, tag="resld",
                                          name="resld", bufs=4)
                            nc.sync.dma_start(
                                t[:], res_dram[(it + 2) * P:(it + 3) * P,
                                               oh * 512:(oh + 1) * 512])
                            resld[(it + 2, oh)] = t
                        ps_o = opsum.tile([P, 512], F32, tag="po", name="ps_o")
                        for hb in range(KB):
                            nc.tensor.matmul(
                                ps_o[:],
                                ct_sb[:, hb, it * P:(it + 1) * P],
                                wosl[hb][:, oh * 512:(oh + 1) * 512],
                                start=(hb == 0), stop=(hb == KB - 1),
                            )
                        o_t = work.tile([P, 512], F32, tag="o_t", name="o_t",
                                        bufs=2)
                        nc.vector.tensor_add(out=o_t[:], in0=ps_o[:],
                                             in1=resld.pop((it, oh))[:])
                        nc.sync.dma_start(
                            out_h[:][it * P:(it + 1) * P,
                                     oh * 512:(oh + 1) * 512],
                            o_t[:])


_CACHE = {}


def get_nc():
    if "nc" not in _CACHE:
        _CACHE["nc"] = build_nc()
    return _CACHE["nc"]


def make_in_maps(inputs):
    x = np.asarray(inputs["hidden_states"], dtype=np.float32)
    wq = np.asarray(inputs["wq"], dtype=np.float32)
    wk = np.asarray(inputs["wk"], dtype=np.float32)
    wv = np.asarray(inputs["wv"], dtype=np.float32)
    wo = np.asarray(inputs["wo"], dtype=np.float32)
    bo = np.asarray(inputs["bo"], dtype=np.float32)
    ln_w = np.asarray(inputs["ln_w"], dtype=np.float32)
    ln_b = np.asarray(inputs["ln_b"], dtype=np.float32)

    bf = ml_dtypes.bfloat16
    wqT = np.ascontiguousarray(wq.T).astype(bf)
    wkT = np.ascontiguousarray(wk.T).astype(bf)
    wvT = np.ascontiguousarray(wv.T).astype(bf)
    woT = np.ascontiguousarray(wo.T).astype(bf)
    lnbbo = (ln_b + bo).astype(np.float32)

    # selector for the reciprocal broadcast: sel[p, hb, m] = 1 iff head p owns
    # row m of h-block hb in the ct layout (even head -> rows 0-63, odd -> 64+)
    sel = np.zeros((NH, KB, P), dtype=np.float32)
    for hb in range(KB):
        sel[2 * hb, hb, :HD] = 1.0
        sel[2 * hb + 1, hb, HD:] = 1.0
    sel = sel.astype(bf)

    # triangle masks in [key, query] layout (see _body)
    r = np.arange(P)[:, None]
    c = np.arange(P)[None, :]
    mask2 = np.stack([(r > c), (r <= c)], axis=1).astype(bf)  # [P, 2, P]

    in_maps = []
    for core in range(NCORES):
        b, hh = divmod(core, 2)
        start = hh * SQ
        xkv = np.zeros((SE, H), dtype=np.float32)
        xkv[WIN:] = x[b, start:start + SQ]
        vones = np.ones(SE, dtype=np.float32)
        if start > 0:
            xkv[:WIN] = x[b, start - WIN:start]
        else:
            vones[:WIN] = 0.0
        in_maps.append({
            "xq": np.ascontiguousarray(x[b, start:start + SQ]),
            "xT": np.ascontiguousarray(xkv.T).astype(bf),
            "wqT": wqT, "wkT": wkT, "wvT": wvT, "woT": woT,
            "lnw": ln_w, "lnbbo": lnbbo,
            "vones": vones.astype(bf),
            "sel": sel,
            "mask2": mask2,
        })
    return in_maps


def kernel(**inputs):
    from concourse.bass_utils import run_bass_kernel_spmd
    nc = get_nc()
    in_maps = make_in_maps(inputs)
    res = run_bass_kernel_spmd(nc, in_maps, core_ids=list(range(NCORES)))
    out = np.empty((B, S, H), dtype=np.float32)
    for core in range(NCORES):
        b, hh = divmod(core, 2)
        out[b, hh * SQ:(hh + 1) * SQ, :] = res.results[core]["out"]
    return out


# revision 15
# speedup vs baseline: 1.2743x; 1.0365x over previous
"""Trainium2 Bass kernel for AnyGPT local-attention block (8 NeuronCores).

Sharding: (batch, seq-half) -> 8 shards of 1024 query tokens each; every core
gets a 256-token k/v halo (zero-padded at sequence start), so no collectives
are needed and the host gather is a pure concatenation.

Per-core pipeline (all matmuls in bf16, LayerNorm/softmax math in fp32):
  qT/kT = W^T-major projections ([H, tok] layout), v natural ([tok, H]) with a
  built-in ones column for softmax denominators; attention uses DIAGONAL
  128-query tiling: each 128-query tile attends exactly 3 aligned 128-key
  blocks (strict-lower triangle / full / upper-incl triangle in [key, query]
  layout), so only 384 scores per query are computed (vs 512 for 256-query
  tiling) and the middle block needs no mask. Units of 4 heads amortize
  per-instruction overhead: 12 score matmuls (parity-paired on PE row
  groups), one EXP (Scalar), one triangle-mask multiply (DVE), 12 ctx
  matmuls, two eviction casts (DVE + GpSimd) and one denominator DMA.
  Softmax is unnormalized (no max subtraction; scores are O(30)) with the
  denominator recovered from the ones row and divided into ctx via a rank-1
  selector-matmul broadcast before the output projection.
"""

import numpy as np
import ml_dtypes

import concourse.bass as bass
import concourse.mybir as mybir
import concourse.tile as tile
from concourse import bacc

F32 = mybir.dt.float32
BF16 = mybir.dt.bfloat16

B, S, H, NH, HD, WIN = 4, 2048, 1024, 16, 64, 256
P = 128
SQ = 1024          # queries per core
SE = SQ + WIN      # ext tokens (halo + queries)
KB = H // P        # 8 contraction blocks
NQT = SQ // P      # 8 query tiles of 128
LN_EPS = 1e-7
NCORES = 8

AF = mybir.ActivationFunctionType
ALU = mybir.AluOpType


def _bcast_ap(handle, n_part):
    """[D] DRAM vector -> [n_part, D] partition-broadcast AP (step 0)."""
    ap = handle[:]
    return bass.AP(tensor=ap.tensor, offset=ap.offset, ap=[[0, n_part]] + list(ap.ap))


def _dead_dim(ap, n, axis=1):
    """Insert a step-0 (broadcast) dim of size n at free-dim position axis."""
    dims = [list(d) for d in ap.ap]
    return bass.AP(tensor=ap.tensor, offset=ap.offset,
                   ap=dims[:axis] + [[0, n]] + dims[axis:])


def build_nc():
    nc = bacc.Bacc("TRN2", target_bir_lowering=False, debug=False)

    xq_h = nc.declare_dram_parameter("xq", [SQ, H], F32, isOutput=False)
    xT_h = nc.declare_dram_parameter("xT", [H, SE], BF16, isOutput=False)
    wqT_h = nc.declare_dram_parameter("wqT", [H, H], BF16, isOutput=False)
    wkT_h = nc.declare_dram_parameter("wkT", [H, H], BF16, isOutput=False)
    wvT_h = nc.declare_dram_parameter("wvT", [H, H], BF16, isOutput=False)
    woT_h = nc.declare_dram_parameter("woT", [H, H], BF16, isOutput=False)
    lnw_h = nc.declare_dram_parameter("lnw", [H], F32, isOutput=False)
    lnbbo_h = nc.declare_dram_parameter("lnbbo", [H], F32, isOutput=False)
    vones_h = nc.declare_dram_parameter("vones", [SE], BF16, isOutput=False)
    sel_h = nc.declare_dram_parameter("sel", [NH, KB, P], BF16, isOutput=False)
    mask2_h = nc.declare_dram_parameter("mask2", [P, 4, 2, P], BF16,
                                        isOutput=False)
    out_h = nc.declare_dram_parameter("out", [SQ, H], F32, isOutput=True)

    with tile.TileContext(nc) as tc:
        _body(tc, nc, xq_h, xT_h, wqT_h, wkT_h, wvT_h, woT_h, lnw_h, lnbbo_h,
              vones_h, sel_h, mask2_h, out_h)
    nc.compile()
    return nc


def _body(tc, nc, xq_h, xT_h, wqT_h, wkT_h, wvT_h, woT_h, lnw_h, lnbbo_h,
          vones_h, sel_h, mask2_h, out_h):
    with (
        tc.tile_pool(name="const", bufs=1) as const,
        tc.tile_pool(name="big", bufs=1) as big,
        tc.tile_pool(name="wpool", bufs=20) as wpool,
        tc.tile_pool(name="work", bufs=3) as work,
        tc.tile_pool(name="lnpool", bufs=2) as lnpool,
    ):
        # residual parks in DRAM between LN (early) and the final add (late);
        # SBUF is too tight to hold 4 MB of fp32 for the whole kernel
        res_dram = nc.dram_tensor("res_dram", [SQ, H], F32)
        # ---- constants ----
        lnw_b = const.tile([P, H], F32)
        nc.sync.dma_start(lnw_b[:], _bcast_ap(lnw_h, P))
        lnbbo_b = const.tile([P, H], F32)
        nc.sync.dma_start(lnbbo_b[:], _bcast_ap(lnbbo_h, P))
        eps_t = const.tile([P, 1], F32)
        nc.vector.memset(eps_t[:], LN_EPS)
        sel_sb = const.tile([NH, KB, P], BF16)
        nc.sync.dma_start(sel_sb[:], sel_h[:][:, :, :])
        # triangle masks in [key, query] layout: slot 0 = strictly-lower
        # (key > query, for the jb=0 block), slot 1 = upper-incl (key <=
        # query, for the jb=2 block); the jb=1 block is fully in-band
        mask2_sb = const.tile([P, 4, 2, P], BF16)
        nc.sync.dma_start(mask2_sb[:], mask2_h[:][:, :, :, :])

        # ---- x^T resident [128, kb, tok]; the first q-proj PSUM group only
        # needs wq cols 0:128 + xt cols WIN:WIN+512, so those DMAs go first
        # and the first matmul starts several us earlier ----
        xt_sb = big.tile([P, KB, SE], BF16, tag="xtr")
        wq_sl = [wpool.tile([P, H], BF16, tag="wslice", name=f"wq_{kb}")
                 for kb in range(KB)]
        for kb in range(KB):
            nc.sync.dma_start(xt_sb[:, kb, WIN:WIN + 512],
                              xT_h[:][kb * P:(kb + 1) * P, WIN:WIN + 512])
            nc.sync.dma_start(wq_sl[kb][:], wqT_h[:][kb * P:(kb + 1) * P, :])
        for kb in range(KB):
            nc.sync.dma_start(xt_sb[:, kb, :WIN],
                              xT_h[:][kb * P:(kb + 1) * P, :WIN])
            nc.sync.dma_start(xt_sb[:, kb, WIN + 512:],
                              xT_h[:][kb * P:(kb + 1) * P, WIN + 512:])

        qT_sb = big.tile([P, KB, SQ], BF16)    # q^T  [H, 1024]
        kT_sb = big.tile([P, KB, SE], BF16)    # k^T  [H, 1280]
        # v natural + a "ones" column that is 0.0 for zero-padded halo tokens,
        # so pad keys contribute exactly nothing to ctx or the denominators
        v_sb = big.tile([P, SE // P, NH, HD + 1], BF16)
        vo = vones_h[:]
        vo_pt = bass.AP(tensor=vo.tensor, offset=vo.offset,
                        ap=[[1, P], [P, SE // P]])
        for h in range(NH):
            nc.sync.dma_start(v_sb[:, :, h, HD], vo_pt)
        ct_sb = big.tile([P, KB, SQ], BF16)    # UNnormalized ctx^T [H, 1024]
        den_sb = big.tile([NH, SQ], F32)       # softmax denominators [head, i]
        recip_sb = big.tile([NH, SQ], BF16)    # 1/den, bulk-reciprocated

        with tc.tile_pool(name="ppsum", bufs=4, space="PSUM") as ppsum:
            # wk slices requested right behind wq/xt so the DMA queues have
            # them in flight well before the k-projection starts
            wk_sl = [wpool.tile([P, H], BF16, tag="wslice", name=f"wk_{kb}")
                     for kb in range(KB)]
            for kb in range(KB):
                nc.sync.dma_start(wk_sl[kb][:], wkT_h[:][kb * P:(kb + 1) * P, :])

            # ---- transposed projections: q^T, k^T ----
            for (w_h, dst, tok0, ntok, wsl) in ((wqT_h, qT_sb, WIN, SQ, wq_sl),
                                                (wkT_h, kT_sb, 0, SE, wk_sl)):
                chunks = [(i, min(512, ntok - i)) for i in range(0, ntok, 512)]
                for ob in range(KB):
                    for (i0, ilen) in chunks:
                        ps = ppsum.tile([P, 512], F32, tag="pj", name="ps_qk")
                        for kb in range(KB):
                            nc.tensor.matmul(
                                ps[:, :ilen],
                                wsl[kb][:, ob * P:(ob + 1) * P],
                                xt_sb[:, kb, tok0 + i0: tok0 + i0 + ilen],
                                start=(kb == 0), stop=(kb == KB - 1),
                            )
                        nc.scalar.copy(out=dst[:, ob, i0:i0 + ilen],
                                       in_=ps[:, :ilen])

            # ---- natural projection: v ----
            wsl = [wpool.tile([P, H], BF16, tag="wslice", name=f"wv_{kb}")
                   for kb in range(KB)]
            for kb in range(KB):
                nc.sync.dma_start(wsl[kb][:], wvT_h[:][kb * P:(kb + 1) * P, :])
            for tt in range(SE // P):
                for oh in range(2):
                    ps = ppsum.tile([P, 512], F32, tag="pj", name="ps_v")
                    for kb in range(KB):
                        nc.tensor.matmul(
                            ps[:],
                            xt_sb[:, kb, tt * P:(tt + 1) * P],
                            wsl[kb][:, oh * 512:(oh + 1) * 512],
                            start=(kb == 0), stop=(kb == KB - 1),
                        )
                    nc.scalar.copy(
                        out=v_sb[:, tt, oh * 8:(oh + 1) * 8, 0:HD],
                        in_=ps[:].rearrange("p (h d) -> p h d", d=HD),
                    )

            # ---- wo slices (prefetch; consumed at the end) ----
            wosl = [wpool.tile([P, H], BF16, tag="wslice", name=f"wo_{kb}")
                    for kb in range(KB)]
            for kb in range(KB):
                nc.sync.dma_start(wosl[kb][:], woT_h[:][kb * P:(kb + 1) * P, :])

            # ---- LayerNorm residual (DVE+GpSimd are idle during
            # projections; the affine tail is split across both) ----
            for it in range(KB):
                x_t = lnpool.tile([P, H], F32, tag="x_t", name="x_t")
                nc.sync.dma_start(x_t[:], xq_h[:][it * P:(it + 1) * P, :])
                stats = lnpool.tile([P, 2, 6], F32, tag="stats", name="stats")
                for g in range(2):
                    nc.vector.bn_stats(out=stats[:, g, :],
                                       in_=x_t[:, g * 512:(g + 1) * 512])
                mv = lnpool.tile([P, 2], F32, tag="mv", name="mv")
                nc.vector.bn_aggr(out=mv[:], in_=stats[:])
                std = lnpool.tile([P, 1], F32, tag="std", name="std")
                nc.scalar.activation(out=std[:], in_=mv[:, 1:2], func=AF.Sqrt,
                                     bias=eps_t[:])
                rstd = lnpool.tile([P, 1], F32, tag="rstd", name="rstd")
                nc.vector.reciprocal(out=rstd[:], in_=std[:])
                res_t = lnpool.tile([P, H], F32, tag="res_t", name="res_t")
                nc.vector.tensor_scalar(out=res_t[:], in0=x_t[:],
                                        scalar1=mv[:, 0:1], scalar2=rstd[:],
                                        op0=ALU.subtract, op1=ALU.mult)
                nc.vector.tensor_mul(out=res_t[:], in0=res_t[:], in1=lnw_b[:])
                nc.vector.tensor_add(out=res_t[:], in0=res_t[:], in1=lnbbo_b[:])
                nc.sync.dma_start(res_dram[it * P:(it + 1) * P, :], res_t[:])

        # ---- attention: diagonal 128-query tiling, 4 heads per unit ----
        # Query tile t attends exactly key blocks t, t+1, t+2 of the ext
        # sequence (jb=0 strict-lower triangle, jb=1 full, jb=2 upper-incl
        # triangle in [key, query] layout). Score matmuls parity-pair on PE
        # row groups (even head rows 0-63, odd rows 64-127). Software
        # pipeline: scores lookahead 2 units, exp lookahead 1.
        with (
            tc.tile_pool(name="spsum", bufs=2, space="PSUM") as spsum,
            tc.tile_pool(name="cpsum", bufs=2, space="PSUM") as cpsum,
        ):
            units = [(t, hq) for t in range(NQT) for hq in range(4)]
            sp_of, probs_of = {}, {}

            def emit_scores(i):
                t, hq = units[i]
                # one 2-bank PSUM tile per head-PAIR, parity stride = one
                # full bank: tile-position-paired matmuls drain concurrently
                # and concurrent drains into one bank are fatal. jb dim is
                # padded 3->4 so each parity owns exactly one bank.
                spA = spsum.tile([P, 2, 4, P], F32, tag="scA", name="spA")
                spB = spsum.tile([P, 2, 4, P], F32, tag="scB", name="spB",
                                 bufs=1)
                for jb in range(3):
                    ks = slice((t + jb) * P, (t + jb + 1) * P)
                    for p in range(4):
                        hb = 2 * hq + p // 2
                        ho = (p % 2) * HD
                        sp = spA if p < 2 else spB
                        nc.tensor.matmul(
                            sp[:, p % 2, jb, :],
                            kT_sb[ho:ho + HD, hb, ks],
                            qT_sb[ho:ho + HD, hb, t * P:(t + 1) * P],
                            start=True, stop=True,
                        )
                sp_of[i] = (spA, spB)

            def emit_probs(i):
                spA, spB = sp_of.pop(i)
                probs = work.tile([P, 4, 3, P], BF16, tag="probs",
                                  name="probs", bufs=3)
                nc.scalar.activation(out=probs[:, 2:4, :, :],
                                     in_=spB[:, :, 0:3, :], func=AF.Exp)
                nc.scalar.activation(out=probs[:, 0:2, :, :],
                                     in_=spA[:, :, 0:3, :], func=AF.Exp)
                # triangle masks on the jb=0 / jb=2 blocks in one multiply;
                # GpSimd (SBUF-to-SBUF, so legal there) — DVE is saturated
                # with PSUM evictions and GpSimd cannot read PSUM
                nc.vector.tensor_mul(
                    out=probs[:, :, 0:3:2, :], in0=probs[:, :, 0:3:2, :],
                    in1=mask2_sb[:])
                probs_of[i] = probs

            pc_of = {}

            def emit_ctx(i):
                t, hq = units[i]
                probs = probs_of.pop(i)
                # one ctx PSUM tile per PAIR of units (8 heads): halves the
                # eviction/staging op count, whose fixed per-op cost
                # dominates DVE time
                if i % 2 == 0:
                    pc_of[i // 2] = cpsum.tile([HD + 1, 8, P], F32, tag="cx",
                                               name="pc", bufs=1)
                pc = pc_of[i // 2]
                for p in range(4):
                    for jb in range(3):
                        nc.tensor.matmul(
                            pc[:, 4 * (hq % 2) + p, :],
                            v_sb[:, t + jb, 4 * hq + p, :],
                            probs[:, p, jb, :],
                            start=(jb == 0), stop=(jb == 2),
                        )
                if hq % 2 == 0:
                    return
                tq = slice(t * P, (t + 1) * P)
                h0 = 4 * (hq - 1)  # first head of this 8-head group
                pc = pc_of.pop(i // 2)
                # evictions all on DVE (GpSimd cannot read PSUM)
                nc.vector.tensor_copy(out=ct_sb[0:HD, h0 // 2:h0 // 2 + 4, tq],
                                      in_=pc[0:HD, 0:8:2, :])
                nc.vector.tensor_copy(out=ct_sb[HD:P, h0 // 2:h0 // 2 + 4, tq],
                                      in_=pc[0:HD, 1:8:2, :])
                dstage = work.tile([1, 8, P], F32, tag="dstage",
                                   name="dstage", bufs=3)
                # denominator staging alternates DVE/Scalar to balance load
                if i % 4 == 1:
                    nc.vector.tensor_copy(out=dstage[:],
                                          in_=pc[HD:HD + 1, :, :])
                else:
                    nc.scalar.copy(out=dstage[:], in_=pc[HD:HD + 1, :, :])
                nc.sync.dma_start(out=den_sb[h0:h0 + 8, tq], in_=dstage[:])
                if hq == 3:
                    # all heads of this query tile done: reciprocate its
                    # denominator slice now so out-proj never waits on it
                    with nc.allow_low_precision(
                            reason="softmax denom recip in bf16: 0.4% rel "
                                   "on a 2e-2 budget"):
                        nc.vector.reciprocal(out=recip_sb[:, tq],
                                             in_=den_sb[:, tq])

            emit_scores(0)
            emit_scores(1)
            emit_probs(0)
            for i in range(len(units)):
                if i + 2 < len(units):
                    emit_scores(i + 2)
                if i + 1 < len(units):
                    emit_probs(i + 1)
                emit_ctx(i)

        # ---- normalize ctx^T, then output projection + residual ----
        # R = selector-matmul broadcast of the per-head reciprocals into the
        # [128, 128] block layout of ct_sb (rows 0-63 <- even head, 64-127 <-
        # odd head), then ct_sb *= R in place.
        with (
            tc.tile_pool(name="opsum", bufs=4, space="PSUM") as opsum,
            tc.tile_pool(name="rpsum", bufs=4, space="PSUM") as rpsum,
        ):
            # R broadcast in bulk: 2 big selector-matmuls per h-block, evicted
            # to SBUF by the otherwise-idle scalar engine, then wide bf16 DVE
            # multiplies normalize ct in place (no per-tile PSUM chain).
            r_sb = big.tile([P, KB, SQ], BF16, tag="xtr")
            resld = {}
            for it in range(2):
                for oh in range(2):
                    t = work.tile([P, 512], F32, tag="resld", name="resld",
                                  bufs=4)
                    nc.sync.dma_start(t[:], res_dram[it * P:(it + 1) * P,
                                                     oh * 512:(oh + 1) * 512])
                    resld[(it, oh)] = t
            for ih in range(2):
                # normalize ct for this i-half first (R broadcast + wide DVE
                # muls), then immediately run its 4 out-proj row-tiles
                hsl = slice(ih * 512, (ih + 1) * 512)
                for hb in range(KB):
                    ps_rb = rpsum.tile([P, 512], F32, tag="rb", name="ps_rb")
                    nc.tensor.matmul(ps_rb[:], sel_sb[:, hb, :],
                                     recip_sb[:, hsl], start=True, stop=True)
                    nc.scalar.copy(out=r_sb[:, hb, hsl], in_=ps_rb[:])
                    nc.vector.tensor_mul(out=ct_sb[:, hb, hsl],
                                         in0=ct_sb[:, hb, hsl],
                                         in1=r_sb[:, hb, hsl])
                for it in range(4 * ih, 4 * ih + 4):
                    for oh in range(2):
                        if it + 2 < KB:  # prefetch 2 row-tiles ahead
                            t = work.tile([P, 512], F32, tag="resld",
                                          name="resld", bufs=4)
                            nc.sync.dma_start(
                                t[:], res_dram[(it + 2) * P:(it + 3) * P,
                                               oh * 512:(oh + 1) * 512])
                            resld[(it + 2, oh)] = t
                        ps_o = opsum.tile([P, 512], F32, tag="po", name="ps_o")
                        for hb in range(KB):
                            nc.tensor.matmul(
                                ps_o[:],
                                ct_sb[:, hb, it * P:(it + 1) * P],
                                wosl[hb][:, oh * 512:(oh + 1) * 512],
                                start=(hb == 0), stop=(hb == KB - 1),
                            )
                        o_t = work.tile([P, 512], F32, tag="o_t", name="o_t",
                                        bufs=2)
                        nc.vector.tensor_add(out=o_t[:], in0=ps_o[:],
                                             in1=resld.pop((it, oh))[:])
                        nc.sync.dma_start(
                            out_h[:][it * P:(it + 1) * P,
                                     oh * 512:(oh + 1) * 512],
                            o_t[:])


_CACHE = {}


def get_nc():
    if "nc" not in _CACHE:
        _CACHE["nc"] = build_nc()
    return _CACHE["nc"]


def make_in_maps(inputs):
    x = np.asarray(inputs["hidden_states"], dtype=np.float32)
    wq = np.asarray(inputs["wq"], dtype=np.float32)
    wk = np.asarray(inputs["wk"], dtype=np.float32)
    wv = np.asarray(inputs["wv"], dtype=np.float32)
    wo = np.asarray(inputs["wo"], dtype=np.float32)
    bo = np.asarray(inputs["bo"], dtype=np.float32)
    ln_w = np.asarray(inputs["ln_w"], dtype=np.float32)
    ln_b = np.asarray(inputs["ln_b"], dtype=np.float32)

    bf = ml_dtypes.bfloat16
    wqT = np.ascontiguousarray(wq.T).astype(bf)
    wkT = np.ascontiguousarray(wk.T).astype(bf)
    wvT = np.ascontiguousarray(wv.T).astype(bf)
    woT = np.ascontiguousarray(wo.T).astype(bf)
    lnbbo = (ln_b + bo).astype(np.float32)

    # selector for the reciprocal broadcast: sel[p, hb, m] = 1 iff head p owns
    # row m of h-block hb in the ct layout (even head -> rows 0-63, odd -> 64+)
    sel = np.zeros((NH, KB, P), dtype=np.float32)
    for hb in range(KB):
        sel[2 * hb, hb, :HD] = 1.0
        sel[2 * hb + 1, hb, HD:] = 1.0
    sel = sel.astype(bf)

    # triangle masks in [key, query] layout (see _body)
    r = np.arange(P)[:, None]
    c = np.arange(P)[None, :]
    m2 = np.stack([(r > c), (r <= c)], axis=1)                # [P, 2, P]
    mask2 = np.broadcast_to(m2[:, None], (P, 4, 2, P)).astype(bf)

    in_maps = []
    for core in range(NCORES):
        b, hh = divmod(core, 2)
        start = hh * SQ
        xkv = np.zeros((SE, H), dtype=np.float32)
        xkv[WIN:] = x[b, start:start + SQ]
        vones = np.ones(SE, dtype=np.float32)
        if start > 0:
            xkv[:WIN] = x[b, start - WIN:start]
        else:
            vones[:WIN] = 0.0
        in_maps.append({
            "xq": np.ascontiguousarray(x[b, start:start + SQ]),
            "xT": np.ascontiguousarray(xkv.T).astype(bf),
            "wqT": wqT, "wkT": wkT, "wvT": wvT, "woT": woT,
            "lnw": ln_w, "lnbbo": lnbbo,
            "vones": vones.astype(bf),
            "sel": sel,
            "mask2": mask2,
        })
    return in_maps


def kernel(**inputs):
    from concourse.bass_utils import run_bass_kernel_spmd
    nc = get_nc()
    in_maps = make_in_maps(inputs)
    res = run_bass_kernel_spmd(nc, in_maps, core_ids=list(range(NCORES)))
    out = np.empty((B, S, H), dtype=np.float32)
    for core in range(NCORES):
        b, hh = divmod(core, 2)
        out[b, hh * SQ:(hh + 1) * SQ, :] = res.results[core]["out"]
    return out


# revision 19
# speedup vs baseline: 1.2930x; 1.0147x over previous
"""Trainium2 Bass kernel for AnyGPT local-attention block (8 NeuronCores).

Sharding: (batch, seq-half) -> 8 shards of 1024 query tokens each; every core
gets a 256-token k/v halo (zero-padded at sequence start), so no collectives
are needed and the host gather is a pure concatenation.

Per-core pipeline (all matmuls in bf16, LayerNorm/softmax math in fp32):
  qT/kT = W^T-major projections ([H, tok] layout), v natural ([tok, H]) with a
  built-in ones column for softmax denominators; attention uses DIAGONAL
  128-query tiling: each 128-query tile attends exactly 3 aligned 128-key
  blocks (strict-lower triangle / full / upper-incl triangle in [key, query]
  layout), so only 384 scores per query are computed (vs 512 for 256-query
  tiling) and the middle block needs no mask. Units of 4 heads amortize
  per-instruction overhead: 12 score matmuls (parity-paired on PE row
  groups), one EXP (Scalar), one triangle-mask multiply (DVE), 12 ctx
  matmuls, two eviction casts (DVE + GpSimd) and one denominator DMA.
  Softmax is unnormalized (no max subtraction; scores are O(30)) with the
  denominator recovered from the ones row and divided into ctx via a rank-1
  selector-matmul broadcast before the output projection.
"""

import numpy as np
import ml_dtypes

import concourse.bass as bass
import concourse.mybir as mybir
import concourse.tile as tile
from concourse import bacc

F32 = mybir.dt.float32
BF16 = mybir.dt.bfloat16
FP8 = mybir.dt.float8e4

B, S, H, NH, HD, WIN = 4, 2048, 1024, 16, 64, 256
P = 128
SQ = 1024          # queries per core
SE = SQ + WIN      # ext tokens (halo + queries)
KB = H // P        # 8 contraction blocks
NQT = SQ // P      # 8 query tiles of 128
LN_EPS = 1e-7
NCORES = 8

AF = mybir.ActivationFunctionType
ALU = mybir.AluOpType


def _bcast_ap(handle, n_part):
    """[D] DRAM vector -> [n_part, D] partition-broadcast AP (step 0)."""
    ap = handle[:]
    return bass.AP(tensor=ap.tensor, offset=ap.offset, ap=[[0, n_part]] + list(ap.ap))


def _dead_dim(ap, n, axis=1):
    """Insert a step-0 (broadcast) dim of size n at free-dim position axis."""
    dims = [list(d) for d in ap.ap]
    return bass.AP(tensor=ap.tensor, offset=ap.offset,
                   ap=dims[:axis] + [[0, n]] + dims[axis:])


def build_nc():
    nc = bacc.Bacc("TRN2", target_bir_lowering=False, debug=False)

    xq_h = nc.declare_dram_parameter("xq", [SQ, H], F32, isOutput=False)
    xT_h = nc.declare_dram_parameter("xT", [H, SE], BF16, isOutput=False)
    xT8_h = nc.declare_dram_parameter("xT8", [H, SE], FP8, isOutput=False)
    wqT_h = nc.declare_dram_parameter("wqT", [H, H], BF16, isOutput=False)
    wkT_h = nc.declare_dram_parameter("wkT", [H, H], BF16, isOutput=False)
    wvT_h = nc.declare_dram_parameter("wvT", [H, H], FP8, isOutput=False)
    woT_h = nc.declare_dram_parameter("woT", [H, H], FP8, isOutput=False)
    lnw_h = nc.declare_dram_parameter("lnw", [H], F32, isOutput=False)
    lnbbo_h = nc.declare_dram_parameter("lnbbo", [H], F32, isOutput=False)
    vones_h = nc.declare_dram_parameter("vones", [SE], BF16, isOutput=False)
    sel_h = nc.declare_dram_parameter("sel", [NH, KB, P], BF16, isOutput=False)
    mask2_h = nc.declare_dram_parameter("mask2", [P, 4, 2, P], BF16,
                                        isOutput=False)
    out_h = nc.declare_dram_parameter("out", [SQ, H], F32, isOutput=True)

    with tile.TileContext(nc) as tc:
        _body(tc, nc, xq_h, xT_h, xT8_h, wqT_h, wkT_h, wvT_h, woT_h, lnw_h,
              lnbbo_h, vones_h, sel_h, mask2_h, out_h)
    nc.compile()
    return nc


def _body(tc, nc, xq_h, xT_h, xT8_h, wqT_h, wkT_h, wvT_h, woT_h, lnw_h,
          lnbbo_h, vones_h, sel_h, mask2_h, out_h):
    with (
        tc.tile_pool(name="const", bufs=1) as const,
        tc.tile_pool(name="big", bufs=1) as big,
        tc.tile_pool(name="wpool", bufs=16) as wpool,
        tc.tile_pool(name="work", bufs=3) as work,
        tc.tile_pool(name="lnpool", bufs=2) as lnpool,
    ):
        # residual parks in DRAM between LN (early) and the final add (late);
        # SBUF is too tight to hold 4 MB of fp32 for the whole kernel
        res_dram = nc.dram_tensor("res_dram", [SQ, H], F32)
        # ---- constants ----
        lnw_b = const.tile([P, H], F32)
        nc.sync.dma_start(lnw_b[:], _bcast_ap(lnw_h, P))
        lnbbo_b = const.tile([P, H], F32)
        nc.sync.dma_start(lnbbo_b[:], _bcast_ap(lnbbo_h, P))
        eps_t = const.tile([P, 1], F32)
        nc.vector.memset(eps_t[:], LN_EPS)
        sel_sb = const.tile([NH, KB, P], BF16)
        nc.sync.dma_start(sel_sb[:], sel_h[:][:, :, :])
        # triangle masks in [key, query] layout: slot 0 = strictly-lower
        # (key > query, for the jb=0 block), slot 1 = upper-incl (key <=
        # query, for the jb=2 block); the jb=1 block is fully in-band
        mask2_sb = const.tile([P, 4, 2, P], BF16)
        nc.sync.dma_start(mask2_sb[:], mask2_h[:][:, :, :, :])

        # ---- x^T resident [128, kb, tok]; the first q-proj PSUM group only
        # needs wq cols 0:128 + xt cols WIN:WIN+512, so those DMAs go first
        # and the first matmul starts several us earlier ----
        xt_sb = big.tile([P, KB, SE], BF16, tag="xtr")
        wq_sl = [wpool.tile([P, H], BF16, tag="wslice", name=f"wq_{kb}")
                 for kb in range(KB)]
        for kb in range(KB):
            nc.sync.dma_start(xt_sb[:, kb, WIN:WIN + 512],
                              xT_h[:][kb * P:(kb + 1) * P, WIN:WIN + 512])
            nc.sync.dma_start(wq_sl[kb][:], wqT_h[:][kb * P:(kb + 1) * P, :])
        for kb in range(KB):
            nc.sync.dma_start(xt_sb[:, kb, :WIN],
                              xT_h[:][kb * P:(kb + 1) * P, :WIN])
            nc.sync.dma_start(xt_sb[:, kb, WIN + 512:],
                              xT_h[:][kb * P:(kb + 1) * P, WIN + 512:])

        # fp8 copy of x^T for the DoubleRow v-projection
        xt8_sb = big.tile([P, KB, SE], FP8)
        for kb in range(KB):
            nc.sync.dma_start(xt8_sb[:, kb, :], xT8_h[:][kb * P:(kb + 1) * P, :])
        qT_sb = big.tile([P, KB, SQ], BF16, tag="qt")   # q^T  [H, 1024]
        kT_sb = big.tile([P, KB, SE], BF16)    # k^T  [H, 1280]
        # v natural + a "ones" column that is 0.0 for zero-padded halo tokens,
        # so pad keys contribute exactly nothing to ctx or the denominators
        v_sb = big.tile([P, SE // P, NH, HD + 1], BF16)
        vo = vones_h[:]
        vo_pt = bass.AP(tensor=vo.tensor, offset=vo.offset,
                        ap=[[1, P], [P, SE // P]])
        for h in range(NH):
            nc.sync.dma_start(v_sb[:, :, h, HD], vo_pt)
        ct_sb = big.tile([P, KB, SQ], BF16)    # UNnormalized ctx^T [H, 1024]
        # normalized fp8 ctx^T aliases qT (dead once attention scores end)
        ct8_sb = big.tile([P, KB, SQ], FP8, tag="qt")
        den_sb = big.tile([NH, SQ], F32)       # softmax denominators [head, i]
        recip_sb = big.tile([NH, SQ], BF16)    # 1/den, bulk-reciprocated

        with tc.tile_pool(name="ppsum", bufs=4, space="PSUM") as ppsum:
            # wk slices requested right behind wq/xt so the DMA queues have
            # them in flight well before the k-projection starts
            wk_sl = [wpool.tile([P, H], BF16, tag="wslice", name=f"wk_{kb}")
                     for kb in range(KB)]
            for kb in range(KB):
                nc.sync.dma_start(wk_sl[kb][:], wkT_h[:][kb * P:(kb + 1) * P, :])

            # ---- transposed projections: q^T, k^T ----
            for (w_h, dst, tok0, ntok, wsl) in ((wqT_h, qT_sb, WIN, SQ, wq_sl),
                                                (wkT_h, kT_sb, 0, SE, wk_sl)):
                chunks = [(i, min(512, ntok - i)) for i in range(0, ntok, 512)]
                for ob in range(KB):
                    for (i0, ilen) in chunks:
                        ps = ppsum.tile([P, 512], F32, tag="pj", name="ps_qk")
                        for kb in range(KB):
                            nc.tensor.matmul(
                                ps[:, :ilen],
                                wsl[kb][:, ob * P:(ob + 1) * P],
                                xt_sb[:, kb, tok0 + i0: tok0 + i0 + ilen],
                                start=(kb == 0), stop=(kb == KB - 1),
                            )
                        nc.scalar.copy(out=dst[:, ob, i0:i0 + ilen],
                                       in_=ps[:, :ilen])

            # ---- natural projection: v ----
            wv8 = big.tile([P, KB, H], FP8)
            for kb in range(KB):
                nc.sync.dma_start(wv8[:, kb, :], wvT_h[:][kb * P:(kb + 1) * P, :])
            for tt in range(SE // P):
                for oh in range(2):
                    ps = ppsum.tile([P, 512], F32, tag="pj", name="ps_v")
                    for j in range(KB // 2):
                        nc.tensor.matmul(
                            ps[:],
                            xt8_sb[:, 2 * j:2 * j + 2, tt * P:(tt + 1) * P],
                            wv8[:, 2 * j:2 * j + 2, oh * 512:(oh + 1) * 512],
                            start=(j == 0), stop=(j == KB // 2 - 1),
                            perf_mode=mybir.MatmulPerfMode.DoubleRow,
                        )
                    nc.scalar.copy(
                        out=v_sb[:, tt, oh * 8:(oh + 1) * 8, 0:HD],
                        in_=ps[:].rearrange("p (h d) -> p h d", d=HD),
                    )

            # ---- wo slices (prefetch; consumed at the end) ----
            wo8 = big.tile([P, KB, H], FP8)
            for kb in range(KB):
                nc.sync.dma_start(wo8[:, kb, :], woT_h[:][kb * P:(kb + 1) * P, :])

            # ---- LayerNorm residual (DVE+GpSimd are idle during
            # projections; the affine tail is split across both) ----
            for it in range(KB):
                x_t = lnpool.tile([P, H], F32, tag="x_t", name="x_t")
                nc.sync.dma_start(x_t[:], xq_h[:][it * P:(it + 1) * P, :])
                stats = lnpool.tile([P, 2, 6], F32, tag="stats", name="stats")
                for g in range(2):
                    nc.vector.bn_stats(out=stats[:, g, :],
                                       in_=x_t[:, g * 512:(g + 1) * 512])
                mv = lnpool.tile([P, 2], F32, tag="mv", name="mv")
                nc.vector.bn_aggr(out=mv[:], in_=stats[:])
                std = lnpool.tile([P, 1], F32, tag="std", name="std")
                nc.scalar.activation(out=std[:], in_=mv[:, 1:2], func=AF.Sqrt,
                                     bias=eps_t[:])
                rstd = lnpool.tile([P, 1], F32, tag="rstd", name="rstd")
                nc.vector.reciprocal(out=rstd[:], in_=std[:])
                # LN applied in place over x_t (SBUF is tight)
                nc.vector.tensor_scalar(out=x_t[:], in0=x_t[:],
                                        scalar1=mv[:, 0:1], scalar2=rstd[:],
                                        op0=ALU.subtract, op1=ALU.mult)
                nc.vector.tensor_mul(out=x_t[:], in0=x_t[:], in1=lnw_b[:])
                nc.vector.tensor_add(out=x_t[:], in0=x_t[:], in1=lnbbo_b[:])
                nc.sync.dma_start(res_dram[it * P:(it + 1) * P, :], x_t[:])

        # ---- attention: diagonal 128-query tiling, 4 heads per unit ----
        # Query tile t attends exactly key blocks t, t+1, t+2 of the ext
        # sequence (jb=0 strict-lower triangle, jb=1 full, jb=2 upper-incl
        # triangle in [key, query] layout). Score matmuls parity-pair on PE
        # row groups (even head rows 0-63, odd rows 64-127). Software
        # pipeline: scores lookahead 2 units, exp lookahead 1.
        with (
            tc.tile_pool(name="spsum", bufs=2, space="PSUM") as spsum,
            tc.tile_pool(name="cpsum", bufs=2, space="PSUM") as cpsum,
        ):
            units = [(t, hq) for t in range(NQT) for hq in range(4)]
            sp_of, probs_of = {}, {}

            def emit_scores(i):
                t, hq = units[i]
                # one 2-bank PSUM tile per head-PAIR, parity stride = one
                # full bank: tile-position-paired matmuls drain concurrently
                # and concurrent drains into one bank are fatal. jb dim is
                # padded 3->4 so each parity owns exactly one bank.
                spA = spsum.tile([P, 2, 4, P], F32, tag="scA", name="spA")
                spB = spsum.tile([P, 2, 4, P], F32, tag="scB", name="spB",
                                 bufs=1)
                for jb in range(3):
                    ks = slice((t + jb) * P, (t + jb + 1) * P)
                    for p in range(4):
                        hb = 2 * hq + p // 2
                        ho = (p % 2) * HD
                        sp = spA if p < 2 else spB
                        nc.tensor.matmul(
                            sp[:, p % 2, jb, :],
                            kT_sb[ho:ho + HD, hb, ks],
                            qT_sb[ho:ho + HD, hb, t * P:(t + 1) * P],
                            start=True, stop=True,
                        )
                sp_of[i] = (spA, spB)

            def emit_probs(i):
                spA, spB = sp_of.pop(i)
                probs = work.tile([P, 4, 3, P], BF16, tag="probs",
                                  name="probs", bufs=3)
                nc.scalar.activation(out=probs[:, 2:4, :, :],
                                     in_=spB[:, :, 0:3, :], func=AF.Exp)
                nc.scalar.activation(out=probs[:, 0:2, :, :],
                                     in_=spA[:, :, 0:3, :], func=AF.Exp)
                # triangle masks on the jb=0 / jb=2 blocks in one multiply;
                # GpSimd (SBUF-to-SBUF, so legal there) — DVE is saturated
                # with PSUM evictions and GpSimd cannot read PSUM
                nc.vector.tensor_mul(
                    out=probs[:, :, 0:3:2, :], in0=probs[:, :, 0:3:2, :],
                    in1=mask2_sb[:])
                probs_of[i] = probs

            pc_of = {}

            def emit_ctx(i):
                t, hq = units[i]
                probs = probs_of.pop(i)
                # one ctx PSUM tile per PAIR of units (8 heads): halves the
                # eviction/staging op count, whose fixed per-op cost
                # dominates DVE time
                if i % 2 == 0:
                    pc_of[i // 2] = cpsum.tile([HD + 1, 8, P], F32, tag="cx",
                                               name="pc", bufs=1)
                pc = pc_of[i // 2]
                for p in range(4):
                    for jb in range(3):
                        nc.tensor.matmul(
                            pc[:, 4 * (hq % 2) + p, :],
                            v_sb[:, t + jb, 4 * hq + p, :],
                            probs[:, p, jb, :],
                            start=(jb == 0), stop=(jb == 2),
                        )
                if hq % 2 == 0:
                    return
                tq = slice(t * P, (t + 1) * P)
                h0 = 4 * (hq - 1)  # first head of this 8-head group
                pc = pc_of.pop(i // 2)
                # evictions all on DVE (GpSimd cannot read PSUM)
                nc.vector.tensor_copy(out=ct_sb[0:HD, h0 // 2:h0 // 2 + 4, tq],
                                      in_=pc[0:HD, 0:8:2, :])
                nc.vector.tensor_copy(out=ct_sb[HD:P, h0 // 2:h0 // 2 + 4, tq],
                                      in_=pc[0:HD, 1:8:2, :])
                dstage = work.tile([1, 8, P], F32, tag="dstage",
                                   name="dstage", bufs=3)
                # denominator staging alternates DVE/Scalar to balance load
                if i % 4 == 1:
                    nc.vector.tensor_copy(out=dstage[:],
                                          in_=pc[HD:HD + 1, :, :])
                else:
                    nc.scalar.copy(out=dstage[:], in_=pc[HD:HD + 1, :, :])
                nc.sync.dma_start(out=den_sb[h0:h0 + 8, tq], in_=dstage[:])
                if hq == 3:
                    # all heads of this query tile done: reciprocate its
                    # denominator slice now so out-proj never waits on it
                    with nc.allow_low_precision(
                            reason="softmax denom recip in bf16: 0.4% rel "
                                   "on a 2e-2 budget"):
                        nc.vector.reciprocal(out=recip_sb[:, tq],
                                             in_=den_sb[:, tq])

            emit_scores(0)
            emit_scores(1)
            emit_probs(0)
            for i in range(len(units)):
                if i + 2 < len(units):
                    emit_scores(i + 2)
                if i + 1 < len(units):
                    emit_probs(i + 1)
                emit_ctx(i)

        # ---- normalize ctx^T, then output projection + residual ----
        # R = selector-matmul broadcast of the per-head reciprocals into the
        # [128, 128] block layout of ct_sb (rows 0-63 <- even head, 64-127 <-
        # odd head), then ct_sb *= R in place.
        with (
            tc.tile_pool(name="opsum", bufs=4, space="PSUM") as opsum,
            tc.tile_pool(name="rpsum", bufs=4, space="PSUM") as rpsum,
        ):
            # R broadcast in bulk: 2 big selector-matmuls per h-block, evicted
            # to SBUF by the otherwise-idle scalar engine, then wide bf16 DVE
            # multiplies normalize ct in place (no per-tile PSUM chain).
            r_sb = big.tile([P, KB, SQ], BF16, tag="xtr")
            resld = {}
            for it in range(2):
                for oh in range(2):
                    t = work.tile([P, 512], F32, tag="resld", name="resld",
                                  bufs=3)
                    nc.sync.dma_start(t[:], res_dram[it * P:(it + 1) * P,
                                                     oh * 512:(oh + 1) * 512])
                    resld[(it, oh)] = t
            for ih in range(2):
                # normalize ct for this i-half first (R broadcast + wide DVE
                # muls), then immediately run its 4 out-proj row-tiles
                hsl = slice(ih * 512, (ih + 1) * 512)
                for hb in range(KB):
                    ps_rb = rpsum.tile([P, 512], F32, tag="rb", name="ps_rb")
                    nc.tensor.matmul(ps_rb[:], sel_sb[:, hb, :],
                                     recip_sb[:, hsl], start=True, stop=True)
                    nc.scalar.copy(out=r_sb[:, hb, hsl], in_=ps_rb[:])
                    nc.vector.tensor_mul(out=ct8_sb[:, hb, hsl],
                                         in0=ct_sb[:, hb, hsl],
                                         in1=r_sb[:, hb, hsl])
                for it in range(4 * ih, 4 * ih + 4):
                    for oh in range(2):
                        if it + 2 < KB:  # prefetch 2 row-tiles ahead
                            t = work.tile([P, 512], F32, tag="resld",
                                          name="resld", bufs=3)
                            nc.sync.dma_start(
                                t[:], res_dram[(it + 2) * P:(it + 3) * P,
                                               oh * 512:(oh + 1) * 512])
                            resld[(it + 2, oh)] = t
                        ps_o = opsum.tile([P, 512], F32, tag="po", name="ps_o")
                        for j in range(KB // 2):
                            nc.tensor.matmul(
                                ps_o[:],
                                ct8_sb[:, 2 * j:2 * j + 2, it * P:(it + 1) * P],
                                wo8[:, 2 * j:2 * j + 2, oh * 512:(oh + 1) * 512],
                                start=(j == 0), stop=(j == KB // 2 - 1),
                                perf_mode=mybir.MatmulPerfMode.DoubleRow,
                            )
                        o_t = work.tile([P, 512], F32, tag="o_t", name="o_t",
                                        bufs=2)
                        nc.vector.tensor_add(out=o_t[:], in0=ps_o[:],
                                             in1=resld.pop((it, oh))[:])
                        nc.sync.dma_start(
                            out_h[:][it * P:(it + 1) * P,
                                     oh * 512:(oh + 1) * 512],
                            o_t[:])


_CACHE = {}


def get_nc():
    if "nc" not in _CACHE:
        _CACHE["nc"] = build_nc()
    return _CACHE["nc"]


def make_in_maps(inputs):
    x = np.asarray(inputs["hidden_states"], dtype=np.float32)
    wq = np.asarray(inputs["wq"], dtype=np.float32)
    wk = np.asarray(inputs["wk"], dtype=np.float32)
    wv = np.asarray(inputs["wv"], dtype=np.float32)
    wo = np.asarray(inputs["wo"], dtype=np.float32)
    bo = np.asarray(inputs["bo"], dtype=np.float32)
    ln_w = np.asarray(inputs["ln_w"], dtype=np.float32)
    ln_b = np.asarray(inputs["ln_b"], dtype=np.float32)

    bf = ml_dtypes.bfloat16
    f8 = ml_dtypes.float8_e4m3fn
    wqT = np.ascontiguousarray(wq.T).astype(bf)
    wkT = np.ascontiguousarray(wk.T).astype(bf)
    wvT = np.ascontiguousarray(wv.T).astype(f8)
    woT = np.ascontiguousarray(wo.T).astype(f8)
    lnbbo = (ln_b + bo).astype(np.float32)

    # selector for the reciprocal broadcast: sel[p, hb, m] = 1 iff head p owns
    # row m of h-block hb in the ct layout (even head -> rows 0-63, odd -> 64+)
    sel = np.zeros((NH, KB, P), dtype=np.float32)
    for hb in range(KB):
        sel[2 * hb, hb, :HD] = 1.0
        sel[2 * hb + 1, hb, HD:] = 1.0
    sel = sel.astype(bf)

    # triangle masks in [key, query] layout (see _body)
    r = np.arange(P)[:, None]
    c = np.arange(P)[None, :]
    m2 = np.stack([(r > c), (r <= c)], axis=1)                # [P, 2, P]
    mask2 = np.broadcast_to(m2[:, None], (P, 4, 2, P)).astype(bf)

    in_maps = []
    for core in range(NCORES):
        b, hh = divmod(core, 2)
        start = hh * SQ
        xkv = np.zeros((SE, H), dtype=np.float32)
        xkv[WIN:] = x[b, start:start + SQ]
        vones = np.ones(SE, dtype=np.float32)
        if start > 0:
            xkv[:WIN] = x[b, start - WIN:start]
        else:
            vones[:WIN] = 0.0
        in_maps.append({
            "xq": np.ascontiguousarray(x[b, start:start + SQ]),
            "xT": np.ascontiguousarray(xkv.T).astype(bf),
            "xT8": np.ascontiguousarray(xkv.T).astype(f8),
            "wqT": wqT, "wkT": wkT, "wvT": wvT, "woT": woT,
            "lnw": ln_w, "lnbbo": lnbbo,
            "vones": vones.astype(bf),
            "sel": sel,
            "mask2": mask2,
        })
    return in_maps


def kernel(**inputs):
    from concourse.bass_utils import run_bass_kernel_spmd
    nc = get_nc()
    in_maps = make_in_maps(inputs)
    res = run_bass_kernel_spmd(nc, in_maps, core_ids=list(range(NCORES)))
    out = np.empty((B, S, H), dtype=np.float32)
    for core in range(NCORES):
        b, hh = divmod(core, 2)
        out[b, hh * SQ:(hh + 1) * SQ, :] = res.results[core]["out"]
    return out
